# revision 1
# baseline (speedup 1.0000x reference)
"""ContxE-style temporal KG embedding scoring kernel for Trainium2 (Bass/Tile).

Contract: kernel(**inputs) takes FULL unsharded numpy inputs and returns the
FULL [B] float32 output. Internally shards the batch across 8 NeuronCores
(data-parallel, tables replicated) and runs a Bass/Tile kernel via
run_bass_kernel_spmd.

Math (per batch element b, window W=5, D=512):
  idx[b,w] = d[b]-(4-w), clamped: negatives -> 365
  c/s[b,w,:] = cos/sin(time_table[idx[b,w]])
  h_real = hr*c - hi*s ; h_img = hr*s + hi*c   (same for t)
  4 attention softmaxes over W of <r, rotated>, then weighted sums,
  out = sum|y_r + rr - z_r| + sum|y_i + ri + z_i|

Device-side strategy (per core, B_loc=2048 = 16 tiles of 128):
  - ONE indirect-DMA gather per embedding pair (tables concatenated host-side)
  - cos|sin rows come from a host-precomputed bf16 table with 4 prefix rows
    equal to row 365, so the W-window gather is ONE contiguous 10KB read per
    batch element (clamp semantics fall out of the prefix rows)
  - logits via fused tensor_tensor_reduce over [c|s]-interleaved pairs
  - attention-weighted sums via scalar_tensor_tensor accumulation chains
"""

import sys

if "/opt/trn_rl_repo" not in sys.path:
    sys.path.insert(0, "/opt/trn_rl_repo")

import numpy as np
import ml_dtypes

import concourse.bass as bass
import concourse.bacc as bacc
import concourse.tile as tile
from concourse import mybir
from concourse.bass_utils import run_bass_kernel_spmd

N_CORES = 8
B = 16384
BL = B // N_CORES          # 2048 per core
P = 128
T = BL // P                # 16 tiles per core
D = 512
W = 5
N_ENTITY = 100000
N_RELATION = 256
N_DAYROWS = 367            # time_table rows
PAD_DAY = 365              # negatives clamp to this row

F32 = mybir.dt.float32
BF16 = mybir.dt.bfloat16
I32 = mybir.dt.int32

AF = mybir.ActivationFunctionType
OP = mybir.AluOpType


from concourse._compat import with_exitstack


@with_exitstack
def _emit(ctx, tc, outs, ins):
    """Emit the per-core program. outs/ins are dicts of DRAM APs."""
    nc = tc.nc
    embE2 = ins["embE2"]      # [2*N_ENTITY, D] f32   (real rows then img rows)
    embR2 = ins["embR2"]      # [2*N_RELATION, D] f32
    cs_ext = ins["cs_ext"]    # [370, 2*D] bf16 ([cos|sin], 4 prefix rows = row 365)
    ht_idx = ins["ht_idx"]    # [P, T*4] i32  (h, h+NE, t, t+NE per tile col)
    r_idx = ins["r_idx"]      # [P, T*2] i32
    d_idx = ins["d_idx"]      # [P, T]   i32
    out = outs["out"]         # [P, T] f32

    singles = ctx.enter_context(tc.tile_pool(name="singles", bufs=1))
    gpool = ctx.enter_context(tc.tile_pool(name="g", bufs=3))
    upool = ctx.enter_context(tc.tile_pool(name="u", bufs=2))
    wpool = ctx.enter_context(tc.tile_pool(name="w", bufs=2))
    spool = ctx.enter_context(tc.tile_pool(name="s", bufs=2))

    # --- load index tiles + output accumulator (resident) ---
    sb_ht = singles.tile([P, T * 4], I32)
    sb_r = singles.tile([P, T * 2], I32)
    sb_d = singles.tile([P, T], I32)
    out_acc = singles.tile([P, T], F32)
    nc.sync.dma_start(sb_ht[:], ht_idx[:])
    nc.sync.dma_start(sb_r[:], r_idx[:])
    nc.sync.dma_start(sb_d[:], d_idx[:])

    for t in range(T):
        # ---- gathers ----
        g = gpool.tile([P, 4 * D], BF16, tag="g")      # hr|hi|tr|ti
        rg = gpool.tile([P, 2 * D], BF16, tag="rg")    # rr|ri
        cs = gpool.tile([P, W * 2 * D], BF16, tag="cs")  # per w: [c_w | s_w]

        for c in range(4):
            nc.gpsimd.indirect_dma_start(
                out=g[:, c * D:(c + 1) * D],
                out_offset=None,
                in_=embE2[:],
                in_offset=bass.IndirectOffsetOnAxis(
                    ap=sb_ht[:, t * 4 + c: t * 4 + c + 1], axis=0
                ),
            )
        for c in range(2):
            nc.gpsimd.indirect_dma_start(
                out=rg[:, c * D:(c + 1) * D],
                out_offset=None,
                in_=embR2[:],
                in_offset=bass.IndirectOffsetOnAxis(
                    ap=sb_r[:, t * 2 + c: t * 2 + c + 1], axis=0
                ),
            )
        nc.gpsimd.indirect_dma_start(
            out=cs[:],
            out_offset=None,
            in_=cs_ext[:],
            in_offset=bass.IndirectOffsetOnAxis(ap=sb_d[:, t: t + 1], axis=0),
        )

        hr = g[:, 0 * D:1 * D]
        hi = g[:, 1 * D:2 * D]
        tr = g[:, 2 * D:3 * D]
        ti = g[:, 3 * D:4 * D]
        rr = rg[:, 0 * D:1 * D]
        ri = rg[:, 1 * D:2 * D]

        # ---- u-pairs: coefficient of c | coefficient of s for each logit type
        # type 0 (real,h): [ rr*hr | -rr*hi ]
        # type 1 (img ,h): [ ri*hi |  ri*hr ]
        # type 2 (real,t): [ rr*tr | -rr*ti ]
        # type 3 (img ,t): [ ri*ti |  ri*tr ]
        U = upool.tile([P, 4, 2 * D], BF16, tag="U")
        nrr = spool.tile([P, D], BF16, tag="nrr")
        nc.vector.tensor_scalar(out=nrr[:], in0=rr, scalar1=-1.0, scalar2=None,
                                op0=OP.mult)
        nc.vector.tensor_tensor(out=U[:, 0, 0:D], in0=rr, in1=hr, op=OP.mult)
        nc.vector.tensor_tensor(out=U[:, 0, D:2 * D], in0=nrr[:], in1=hi,
                                op=OP.mult)
        nc.vector.tensor_tensor(out=U[:, 1, 0:D], in0=ri, in1=hi, op=OP.mult)
        nc.vector.tensor_tensor(out=U[:, 1, D:2 * D], in0=ri, in1=hr, op=OP.mult)
        nc.vector.tensor_tensor(out=U[:, 2, 0:D], in0=rr, in1=tr, op=OP.mult)
        nc.vector.tensor_tensor(out=U[:, 2, D:2 * D], in0=nrr[:], in1=ti,
                                op=OP.mult)
        nc.vector.tensor_tensor(out=U[:, 3, 0:D], in0=ri, in1=ti, op=OP.mult)
        nc.vector.tensor_tensor(out=U[:, 3, D:2 * D], in0=ri, in1=tr, op=OP.mult)

        # ---- logits: L[b, ty, w] = sum(U[ty] * cs[w]) ----
        # DVE: one broadcast TT per type over all 5 windows;
        # ACT: per-(ty,w) Copy with accum_out does the reduction.
        L = spool.tile([P, 4 * W], F32, tag="L")
        dummy = spool.tile([P, 2 * D], BF16, tag="dummy")
        csv = cs.rearrange("p (w e) -> p w e", w=W)
        # types 0,1: fused STT dot on DVE; types 2,3: DVE broadcast-mult
        # + ACT accum reduce (balances DVE vs ACT)
        for ty in range(2):
            for w in range(W):
                nc.vector.scalar_tensor_tensor(
                    out=dummy[:], in0=U[:, ty, :], scalar=1.0,
                    in1=csv[:, w, :], op0=OP.mult, op1=OP.mult,
                    accum_out=L[:, ty * W + w: ty * W + w + 1])
        for ty in range(2, 4):
            prod5 = wpool.tile([P, W, 2 * D], BF16, tag="prod5")
            ub = U[:, ty: ty + 1, :].to_broadcast([P, W, 2 * D])
            nc.vector.tensor_tensor(out=prod5[:], in0=ub, in1=csv, op=OP.mult)
            for w in range(W):
                nc.scalar.activation(
                    dummy[:], prod5[:, w, :], AF.Copy,
                    accum_out=L[:, ty * W + w: ty * W + w + 1])

        # ---- softmax over w (logits are O(1); skip max-subtraction) ----
        Ex = spool.tile([P, 4 * W], F32, tag="Ex")
        Sm = spool.tile([P, 4], F32, tag="Sm")
        Rc = spool.tile([P, 4], F32, tag="Rc")
        Al = spool.tile([P, 4 * W], F32, tag="Al")
        nc.scalar.activation(Ex[:], L[:], AF.Exp)
        nc.vector.tensor_reduce(
            out=Sm[:], in_=Ex.rearrange("p (t w) -> p t w", w=W),
            axis=mybir.AxisListType.X, op=OP.add)
        nc.vector.reciprocal(Rc[:], Sm[:])
        for ty in range(4):
            nc.vector.tensor_scalar(
                out=Al[:, ty * W:(ty + 1) * W],
                in0=Ex[:, ty * W:(ty + 1) * W],
                scalar1=Rc[:, ty: ty + 1],
                scalar2=None,
                op0=OP.mult,
            )

        # ---- attention-weighted sums: CSS[ty] = sum_w alpha[ty,w]*cs[w] ----
        # ACT: 5 scaled copies (scale = alpha per partition); DVE: tree-add.
        CSS = wpool.tile([P, 4, 2 * D], BF16, tag="CSS")
        for ty in range(4):
            ap5 = wpool.tile([P, W, 2 * D], BF16, tag="ap5")
            for w in range(W):
                # split scaled copies between ACT and DVE-TS; types 0-1
                # lean more on ACT (their logit dots run on DVE)
                if w < (4 if ty < 2 else 3):
                    nc.scalar.activation(
                        ap5[:, w, :], csv[:, w, :], AF.Copy,
                        scale=Al[:, ty * W + w: ty * W + w + 1])
                else:
                    nc.vector.tensor_scalar(
                        out=ap5[:, w, :], in0=csv[:, w, :],
                        scalar1=Al[:, ty * W + w: ty * W + w + 1],
                        scalar2=None, op0=OP.mult)
            t12 = spool.tile([P, 2, 2 * D], BF16, tag="t12")
            # one wide add: (p0+p2 | p1+p3), then fold halves, then +p4
            nc.vector.tensor_tensor(
                out=t12[:], in0=ap5[:, 0:2, :], in1=ap5[:, 2:4, :], op=OP.add)
            nc.vector.tensor_tensor(out=t12[:, 0, :], in0=t12[:, 0, :],
                                    in1=t12[:, 1, :], op=OP.add)
            nc.vector.tensor_tensor(out=CSS[:, ty, :], in0=t12[:, 0, :],
                                    in1=ap5[:, 4, :], op=OP.add)

        # ---- recombine: y/z vectors [P, D] ----
        # y_r = hr*Cc0 - hi*Cs0 ; y_i = hr*Cs1 + hi*Cc1
        # z_r = tr*Cc2 - ti*Cs2 ; z_i = tr*Cs3 + ti*Cc3
        p1 = spool.tile([P, D], BF16, tag="p1")
        p2 = spool.tile([P, D], BF16, tag="p2")
        yz = wpool.tile([P, 4, D], BF16, tag="yz")
        specs = [
            (hr, CSS[:, 0, 0:D], hi, CSS[:, 0, D:2 * D], OP.subtract),  # y_r
            (hr, CSS[:, 1, D:2 * D], hi, CSS[:, 1, 0:D], OP.add),       # y_i
            (tr, CSS[:, 2, 0:D], ti, CSS[:, 2, D:2 * D], OP.subtract),  # z_r
            (tr, CSS[:, 3, D:2 * D], ti, CSS[:, 3, 0:D], OP.add),       # z_i
        ]
        for k, (a0, b0, a1, b1, opk) in enumerate(specs):
            # y_i/z_r/z_i products+combine go to GpSimd to offload DVE
            eng = nc.vector if k < 1 else nc.gpsimd
            pa = p1 if k < 1 else spool.tile([P, D], BF16, tag=f"gp{k}a")
            pb = p2 if k < 1 else spool.tile([P, D], BF16, tag=f"gp{k}b")
            eng.tensor_tensor(out=pa[:], in0=a0, in1=b0, op=OP.mult)
            eng.tensor_tensor(out=pb[:], in0=a1, in1=b1, op=OP.mult)
            eng.tensor_tensor(out=yz[:, k, :], in0=pa[:], in1=pb[:], op=opk)

        # ---- final: out = sum|y_r + rr - z_r| + sum|y_i + ri + z_i| ----
        f1 = spool.tile([P, D], BF16, tag="f1")
        f2 = spool.tile([P, D], BF16, tag="f2")
        o_r = spool.tile([P, 1], F32, tag="o_r")
        o_i = spool.tile([P, 1], F32, tag="o_i")
        nc.vector.tensor_tensor(out=f1[:], in0=yz[:, 0, :], in1=rr, op=OP.add)
        nc.vector.tensor_tensor(out=f2[:], in0=f1[:], in1=yz[:, 2, :], op=OP.subtract)
        nc.vector.tensor_reduce(
            out=o_r[:], in_=f2[:], axis=mybir.AxisListType.X, op=OP.add,
            apply_absolute_value=True)
        nc.vector.tensor_tensor(out=f1[:], in0=yz[:, 1, :], in1=ri, op=OP.add)
        nc.vector.tensor_tensor(out=f2[:], in0=f1[:], in1=yz[:, 3, :], op=OP.add)
        nc.vector.tensor_reduce(
            out=o_i[:], in_=f2[:], axis=mybir.AxisListType.X, op=OP.add,
            apply_absolute_value=True)
        nc.vector.tensor_tensor(
            out=out_acc[:, t: t + 1], in0=o_r[:], in1=o_i[:], op=OP.add)

    nc.sync.dma_start(out[:], out_acc[:])


def _host_prep(h_i, t_i, r_i, d_i, emb_E_real, emb_E_img, emb_R_real,
               emb_R_img, time_table):
    """Host-side layout prep (cheap index/table manipulation only)."""
    embE2 = np.ascontiguousarray(
        np.concatenate([emb_E_real, emb_E_img], axis=0), dtype=np.float32)
    embR2 = np.ascontiguousarray(
        np.concatenate([emb_R_real, emb_R_img], axis=0), dtype=np.float32)
    tt = np.asarray(time_table, dtype=np.float32)
    cs = np.concatenate([np.cos(tt), np.sin(tt)], axis=1)  # [367, 1024] f32
    # 4 prefix rows equal to row PAD_DAY implement the negative-index clamp;
    # row d of the original table sits at ext row d+4, so a window gather for
    # batch element b is rows d[b] .. d[b]+4 of cs_ext == one contiguous read.
    cs_ext = np.concatenate(
        [np.broadcast_to(cs[PAD_DAY], (4, 2 * D)), cs[:366]], axis=0)
    cs_ext = np.ascontiguousarray(cs_ext, dtype=ml_dtypes.bfloat16)  # [370, 1024]

    ht = np.stack(
        [h_i, h_i + N_ENTITY, t_i, t_i + N_ENTITY], axis=1).astype(np.int32)
    rx = np.stack([r_i, r_i + N_RELATION], axis=1).astype(np.int32)
    dx = d_i.astype(np.int32).reshape(B, 1)

    def tileize(a):
        # [BL, C] -> [P, T*C] with element [p, t*C+c] = a[t*P+p, c]
        C = a.shape[1]
        return np.ascontiguousarray(
            a.reshape(T, P, C).transpose(1, 0, 2).reshape(P, T * C))

    in_maps = []
    for core in range(N_CORES):
        sl = slice(core * BL, (core + 1) * BL)
        in_maps.append(dict(
            embE2=embE2,
            embR2=embR2,
            cs_ext=cs_ext,
            ht_idx=tileize(ht[sl]),
            r_idx=tileize(rx[sl]),
            d_idx=tileize(dx[sl]),
        ))
    return in_maps


def build_nc():
    nc = bacc.Bacc(
        "TRN2",
        target_bir_lowering=False,
        debug=False,
        enable_asserts=False,
        num_devices=N_CORES,
    )
    ins = dict(
        embE2=nc.dram_tensor("embE2", [2 * N_ENTITY, D], F32,
                             kind="ExternalInput").ap(),
        embR2=nc.dram_tensor("embR2", [2 * N_RELATION, D], F32,
                             kind="ExternalInput").ap(),
        cs_ext=nc.dram_tensor("cs_ext", [370, 2 * D], BF16,
                              kind="ExternalInput").ap(),
        ht_idx=nc.dram_tensor("ht_idx", [P, T * 4], I32,
                              kind="ExternalInput").ap(),
        r_idx=nc.dram_tensor("r_idx", [P, T * 2], I32,
                             kind="ExternalInput").ap(),
        d_idx=nc.dram_tensor("d_idx", [P, T], I32,
                             kind="ExternalInput").ap(),
    )
    outs = dict(
        out=nc.dram_tensor("out", [P, T], F32, kind="ExternalOutput").ap(),
    )
    with tile.TileContext(nc) as tc:
        _emit(tc, outs, ins)
    nc.compile()
    return nc


_NC_CACHE = {}


def kernel(h_i, t_i, r_i, d_i, emb_E_real, emb_E_img, emb_R_real, emb_R_img,
           time_table, _want_results=False, _trace=False):
    in_maps = _host_prep(h_i, t_i, r_i, d_i, emb_E_real, emb_E_img,
                         emb_R_real, emb_R_img, time_table)
    if "nc" not in _NC_CACHE:
        _NC_CACHE["nc"] = build_nc()
    nc = _NC_CACHE["nc"]
    res = run_bass_kernel_spmd(
        nc, in_maps, core_ids=list(range(N_CORES)), trace=_trace)
    out = np.empty((B,), np.float32)
    for core in range(N_CORES):
        o = res.results[core]["out"]  # [P, T]
        out[core * BL:(core + 1) * BL] = np.asarray(o).T.reshape(BL)
    if _want_results:
        return out, res
    return out



# revision 28
# speedup vs baseline: 1.8614x; 1.8614x over previous
"""ContxE-style temporal KG embedding scoring kernel for Trainium2 (Bass/Tile).

Contract: kernel(**inputs) takes FULL unsharded numpy inputs and returns the
FULL [B] float32 output. Internally shards the batch across 8 NeuronCores
(data-parallel, tables replicated) and runs a Bass/Tile kernel via
run_bass_kernel_spmd.

Math (per batch element b, window W=5, D=512):
  idx[b,w] = d[b]-(4-w), clamped: negatives -> 365
  c/s[b,w,:] = cos/sin(time_table[idx[b,w]])
  h_real = hr*c - hi*s ; h_img = hr*s + hi*c   (same for t)
  4 attention softmaxes over W of <r, rotated>, then weighted sums,
  out = sum|y_r + rr - z_r| + sum|y_i + ri + z_i|

Key identity: time_table[d] = base + 0.01*d*inc, so row d-k is a per-dim
rotation of row d by angle k*phi (phi = 0.01*inc). The 5-window expansion
collapses into fixed [5,512] basis tables A_k=cos(k*phi), B_k=sin(k*phi):
  c_m = A*c_d + B*s_d ; s_m = A*s_d - B*c_d
  logits L[ty,w] = A_w . Z_ty_a (+/-) B_w . Z_ty_b  (TensorE, d-contraction)
  y/z = (sum_w alpha*A_w) (.) H_R + (sum_w alpha*B_w) (.) H_I
        with coefficient fields from TensorE matmuls (alpha as stationary).
Clamped elements (d<4) are routed host-side to tile 0 and fixed exactly
there with per-d-value basis tables (theta_365 = theta_d + (365-d)*phi is
still a fixed rotation for fixed d) plus predicated selects.

Layout notes: logits live w-major in a [128, P] PSUM tile, attention type
ty occupying rows 32*ty..32*ty+4 (PE tile_position requires 32-aligned
output bases); w-columns of stationaries are zero-padded to 32 so the
whole PSUM tile is written and a single ACT Exp covers it.
"""

import sys

if "/opt/trn_rl_repo" not in sys.path:
    sys.path.insert(0, "/opt/trn_rl_repo")

import numpy as np
import ml_dtypes

import concourse.bass as bass
import concourse.bacc as bacc
import concourse.tile as tile
from concourse import mybir
from concourse.bass_utils import run_bass_kernel_spmd

N_CORES = 8
B = 16384
BL = B // N_CORES          # 2048 per core
P = 128
T = BL // P                # 16 tiles per core
GT = 4                     # tiles per gather group
D = 512
W = 5
N_ENTITY = 100000
N_RELATION = 256
N_DAY = 365                # clamp row for negative indices
ROW_EI = N_ENTITY
ROW_RR = 2 * N_ENTITY
ROW_RI = 2 * N_ENTITY + N_RELATION
ROW_CS = 2 * N_ENTITY + 2 * N_RELATION
N_ROWS = ROW_CS + 2 * (N_DAY + 1)

F32 = mybir.dt.float32
BF16 = mybir.dt.bfloat16
I32 = mybir.dt.int32
U8 = mybir.dt.uint8

AF = mybir.ActivationFunctionType
OP = mybir.AluOpType

from concourse._compat import with_exitstack


@with_exitstack
def _emit(ctx, tc, outs, ins):
    nc = tc.nc
    embM = ins["embM"]        # [N_ROWS, D] bf16 mega-table
    idx = ins["idx"]          # [P, T*4] i32 gather row-pair starts
    ident = ins["ident"]      # [P, P] bf16 identity
    ltab = ins["ltab"]        # [P, 4*3*32] bf16: (chunk, {A,Bp,Bn}, w32)
    ltabk = ins["ltabk"]      # [P, 4*4*3*8] bf16: (k, chunk, {A,Bp,Bn}, w8)
    ctab = ins["ctab"]        # [P, 4096] bf16 coeff rhs (h rows 0-63 cols :2048,
                              #                           z rows 64-127 cols 2048:)
    ctabk = ins["ctabk"]      # [64, 4096] bf16 k-basis coeff rhs (h | z cols)
    gsum = ins["gsum"]        # [P, 4] bf16 group-sum selector
    grep = ins["grep"]        # [4, P] bf16 replicate selector
    lmask = ins["lmask"]      # [P, P] bf16 tile-0 logit select mask (w-major)
    cmask = ins["cmask"]      # [P, 1] bf16 tile-0 coeff select mask
    kmask = ins["kmask"]      # [64, P] bf16 tile-0 k-block alpha mask
    sel = ins["sel"]          # [P, P] bf16 alpha k-stack selector
    out = outs["out"]         # [P, T] f32

    const = ctx.enter_context(tc.tile_pool(name="const", bufs=1))
    gpool = ctx.enter_context(tc.tile_pool(name="g", bufs=2))
    wpool = ctx.enter_context(tc.tile_pool(name="w", bufs=2))
    zpool = ctx.enter_context(tc.tile_pool(name="z", bufs=2))
    ztpp = ctx.enter_context(tc.tile_pool(name="ztp", bufs=2, space="PSUM"))
    cfpp = ctx.enter_context(tc.tile_pool(name="cfp", bufs=2, space="PSUM"))
    lpp = ctx.enter_context(tc.tile_pool(name="lp", bufs=1, space="PSUM"))
    rpp = ctx.enter_context(tc.tile_pool(name="rp", bufs=1, space="PSUM"))

    # --- resident constants ---
    sb_idx = const.tile([P, T * 4], I32)
    sb_id = const.tile([P, P], BF16)
    sb_lt = const.tile([P, 4 * 3 * 32], BF16)
    sb_ltk = const.tile([P, 4 * 4 * 3 * 8], BF16)
    sb_ct = const.tile([P, 4096], BF16)
    sb_ctk = const.tile([64, 4096], BF16)
    sb_gs = const.tile([P, 4], BF16)
    sb_gr = const.tile([4, P], BF16)
    sb_lm = const.tile([P, P], U8)
    sb_cm = const.tile([P, 1], U8)
    sb_km = const.tile([64, P], BF16)
    sb_sel = const.tile([P, P], BF16)
    oacc = const.tile([P, T], F32)
    nc.sync.dma_start(sb_idx[:], idx[:])
    nc.sync.dma_start(sb_id[:], ident[:])
    nc.sync.dma_start(sb_lt[:], ltab[:])
    nc.sync.dma_start(sb_ltk[:], ltabk[:])
    nc.sync.dma_start(sb_ct[:], ctab[:])
    nc.sync.dma_start(sb_ctk[:], ctabk[:])
    nc.sync.dma_start(sb_gs[:], gsum[:])
    nc.sync.dma_start(sb_gr[:], grep[:])
    nc.sync.dma_start(sb_lm[:], lmask[:])
    nc.sync.dma_start(sb_cm[:], cmask[:])
    nc.sync.dma_start(sb_km[:], kmask[:])
    nc.sync.dma_start(sb_sel[:], sel[:])

    ltv = sb_lt.rearrange("p (c v w) -> p c v w", c=4, v=3)
    ltkv = sb_ltk.rearrange("p (k c v w) -> p k c v w", k=4, c=4, v=3)

    for t in range(T):
        gbuf = gpool.tile([P, 4, 2, D], BF16, tag="gath")
        for c in range(4):
            nc.gpsimd.indirect_dma_start(
                out=gbuf[:, c].rearrange("p q d -> p (q d)"),
                out_offset=None,
                in_=embM[:],
                in_offset=bass.IndirectOffsetOnAxis(
                    ap=sb_idx[:, 4 * t + c:4 * t + c + 1], axis=0
                ),
            )
        G = gbuf.rearrange("p c q d -> p (c q) d")   # [P, 8, D]
        csp = G[:, 6:8]                     # [P, 2, D] = [c | s]
        rrri = G[:, 4:6]                    # [P, 2, D] = [rr | ri]

        # ---- [-s | c] helper (ACT) ----
        sce = wpool.tile([P, 2, D], BF16, tag="sce")
        nc.scalar.activation(sce[:, 0], G[:, 7], AF.Copy, scale=-1.0)
        nc.scalar.activation(sce[:, 1], G[:, 6], AF.Copy)

        # ---- rotation: H = [H_R | H_I], T2 = [T_R | T_I] ----
        def dup(ap):
            return ap.rearrange("p (o d) -> p o d", o=1).to_broadcast([P, 2, D])

        mh1 = wpool.tile([P, 2, D], BF16, tag="mh1")
        mh2 = wpool.tile([P, 2, D], BF16, tag="mh2")
        H = wpool.tile([P, 2, D], BF16, tag="H")
        nc.vector.tensor_tensor(out=mh1[:], in0=csp, in1=dup(G[:, 0]), op=OP.mult)
        nc.vector.tensor_tensor(out=mh2[:], in0=sce[:], in1=dup(G[:, 1]), op=OP.mult)
        nc.vector.tensor_tensor(out=H[:], in0=mh1[:], in1=mh2[:], op=OP.add)
        mt1 = wpool.tile([P, 2, D], BF16, tag="mt1")
        mt2 = wpool.tile([P, 2, D], BF16, tag="mt2")
        T2 = wpool.tile([P, 2, D], BF16, tag="T2")
        nc.vector.tensor_tensor(out=mt1[:], in0=csp, in1=dup(G[:, 2]), op=OP.mult)
        nc.gpsimd.tensor_tensor(out=mt2[:], in0=sce[:], in1=dup(G[:, 3]), op=OP.mult)
        nc.gpsimd.tensor_tensor(out=T2[:], in0=mt1[:], in1=mt2[:], op=OP.add)

        # ---- products Z_ty = [a | b] ----
        Z = [zpool.tile([P, 2, D], BF16, tag=f"Z{i}", name=f"Z{i}")
             for i in range(4)]
        nc.vector.tensor_tensor(out=Z[0][:], in0=dup(G[:, 4]), in1=H[:], op=OP.mult)
        nc.vector.tensor_tensor(out=Z[1][:, 0], in0=G[:, 5], in1=H[:, 1], op=OP.mult)
        nc.vector.tensor_tensor(out=Z[1][:, 1], in0=G[:, 5], in1=H[:, 0], op=OP.mult)
        nc.vector.tensor_tensor(out=Z[2][:], in0=dup(G[:, 4]), in1=T2[:], op=OP.mult)
        nc.vector.tensor_tensor(out=Z[3][:, 0], in0=G[:, 5], in1=T2[:, 1], op=OP.mult)
        nc.vector.tensor_tensor(out=Z[3][:, 1], in0=G[:, 5], in1=T2[:, 0], op=OP.mult)

        # ---- transposes: ztsb[ty] [P(d'), 2, 4, P] = (a|b, chunk, b-cols) ----
        ztsb = []
        for ty in range(4):
            ztp = ztpp.tile([P, 2, 4, P], BF16, tag="zt")
            for half in range(2):
                for c in range(4):
                    nc.tensor.transpose(
                        ztp[:, half, c], Z[ty][:, half, c * P:(c + 1) * P],
                        sb_id[:],
                    )
            zs = wpool.tile([P, 2, 4, P], BF16, tag=f"zts{ty}", name=f"zts{ty}")
            if ty % 2 == 0:
                nc.scalar.activation(zs[:], ztp[:], AF.Copy)
            else:
                nc.vector.tensor_copy(zs[:], ztp[:])
            ztsb.append(zs)

        # ---- logits: Lh[32*u+w] = ty 0/1, Lz[32*u+w] = ty 2/3 ----
        L2 = lpp.tile([64, 2, P], F32, tag="L2")
        Lh, Lz = L2[:, 0], L2[:, 1]
        for ty in range(4):
            Lt = Lh if ty < 2 else Lz
            bvar = 1 if ty % 2 == 0 else 2   # Bp / Bn
            r0 = slice(32 * (ty % 2), 32 * (ty % 2) + 32)
            for c in range(4):
                nc.tensor.matmul(Lt[r0], ltv[:, c, 0], ztsb[ty][:, 0, c],
                                 start=(c == 0), stop=False)
            for c in range(4):
                nc.tensor.matmul(Lt[r0], ltv[:, c, bvar], ztsb[ty][:, 1, c],
                                 start=False, stop=(c == 3))

        if t == 0:
            t0pp = ctx.enter_context(
                tc.tile_pool(name="t0p", bufs=1, space="PSUM"))
            t0sb = ctx.enter_context(tc.tile_pool(name="t0s", bufs=1))
            Lk2 = t0pp.tile([64, 2, P], F32, tag="Lk2")
            Lkh, Lkz = Lk2[:, 0], Lk2[:, 1]
            for ty in range(4):
                Lkt = Lkh if ty < 2 else Lkz
                bvar = 1 if ty % 2 == 0 else 2
                r0 = slice(32 * (ty % 2), 32 * (ty % 2) + 8)
                for k in range(4):
                    sl = slice(32 * k, 32 * (k + 1))
                    for c in range(4):
                        nc.tensor.matmul(
                            Lkt[r0, sl], ltkv[:, k, c, 0],
                            ztsb[ty][:, 0, c, sl], start=(c == 0), stop=False)
                    for c in range(4):
                        nc.tensor.matmul(
                            Lkt[r0, sl], ltkv[:, k, c, bvar],
                            ztsb[ty][:, 1, c, sl], start=False, stop=(c == 3))
            for u in range(2):
                r8 = slice(32 * u, 32 * u + 8)
                nc.vector.copy_predicated(Lh[r8], sb_lm[0:64][r8], Lkh[r8])
                nc.vector.copy_predicated(Lz[r8], sb_lm[64:128][r8], Lkz[r8])

        # ---- softmax helpers (w-major) ----
        Ex = wpool.tile([P, P], BF16, tag="Ex")
        nc.scalar.activation(Ex[0:64], Lh, AF.Exp)
        nc.scalar.activation(Ex[64:128], Lz, AF.Exp)
        M2 = rpp.tile([P, 2, P], F32, tag="M2")
        S4 = M2[0:4, 0]
        nc.tensor.matmul(S4, sb_gs[:], Ex[:], start=True, stop=True)
        Rec4 = wpool.tile([4, P], F32, tag="Rec4")
        nc.vector.reciprocal(Rec4[:], S4)
        Rec4b = wpool.tile([4, P], BF16, tag="Rec4b")
        nc.scalar.activation(Rec4b[:], Rec4[:], AF.Copy)
        R = M2[:, 1]
        nc.tensor.matmul(R, sb_gr[:], Rec4b[:], start=True, stop=True)
        aln = wpool.tile([P, P], BF16, tag="aln")
        nc.vector.tensor_tensor(out=aln[:], in0=Ex[:], in1=R, op=OP.mult)

        if t == 0:
            # k-stacked alpha for the clamp-basis coefficient matmuls:
            # row (16k+8u+w) <- aln[32u+w] (h) / aln[64+32u+w] (z), masked to
            # k-block columns. Selection via matmul, reusing the Lk2 psum.
            nc.tensor.matmul(Lk2[:, 0], sb_sel[:, 0:64], aln[:],
                             start=True, stop=True)
            nc.tensor.matmul(Lk2[:, 1], sb_sel[:, 64:128], aln[:],
                             start=True, stop=True)
            alnK = t0sb.tile([64, 2, P], BF16)
            nc.vector.tensor_tensor(
                out=alnK[:], in0=Lk2[:],
                in1=sb_km.rearrange("(q o) b -> q o b", o=1)
                .to_broadcast([64, 2, P]),
                op=OP.mult)

        # ---- coefficient matmuls (bank-sized quarters) + recombination ----
        # quarter q of side: q0 -> uA0*V0, q1 -> uA1*V1, q2 -> uB0*V1,
        # q3 -> uB1'*V0  (V = H for side 0, T2 for side 1)
        fth = wpool.tile([P, 2, D], BF16, tag="fth")
        fm2 = wpool.tile([P, 2, D], BF16, tag="fm2")
        fv = wpool.tile([P, 2, D], BF16, tag="fv")
        cmb = sb_cm.to_broadcast([P, D])
        for side, (Vv, nm) in enumerate(((H, "h"), (T2, "z"))):
            rs = slice(64 * side, 64 * (side + 1))
            m1 = wpool.tile([P, 2, D], BF16, tag=f"m1{nm}", name=f"m1{nm}")
            for q in range(4):
                cfq = cfpp.tile([P, D], F32, tag="cfq")
                nc.tensor.matmul(cfq[:], aln[rs],
                                 sb_ct[rs, (4 * side + q) * D:
                                       (4 * side + q + 1) * D],
                                 start=True, stop=True)
                if t == 0:
                    cfkq = t0pp.tile([P, D], F32, tag="cfkq")
                    nc.tensor.matmul(
                        cfkq[:], alnK[:, side],
                        sb_ctk[:, (4 * side + q) * D:(4 * side + q + 1) * D],
                        start=True, stop=True)
                    nc.vector.copy_predicated(cfq[:], cmb, cfkq[:])
                dst = (m1 if q < 2 else fm2)[:, q % 2]
                vin = Vv[:, (0, 1, 1, 0)[q]]
                nc.vector.tensor_tensor(out=dst, in0=cfq[:], in1=vin,
                                        op=OP.mult)
            if side == 0:
                nc.vector.tensor_tensor(out=fth[:], in0=m1[:], in1=fm2[:],
                                        op=OP.add)
                nc.vector.tensor_tensor(out=fv[:], in0=fth[:], in1=rrri,
                                        op=OP.add)
            else:
                nc.gpsimd.tensor_tensor(out=fth[:], in0=m1[:], in1=fm2[:],
                                        op=OP.add)
                nc.vector.tensor_tensor(out=fv[:], in0=fv[:], in1=fth[:],
                                        op=OP.add)

        # ---- final abs-sum ----
        scr = wpool.tile([P, 2 * D], BF16, tag="scr")
        nc.scalar.activation(scr[:], fv.rearrange("p q d -> p (q d)"), AF.Abs,
                             accum_out=oacc[:, t:t + 1])

    nc.sync.dma_start(out[:], oacc[:])


# ---------------------------------------------------------------------------
# Host-side preparation
# ---------------------------------------------------------------------------

def _build_tables(time_table):
    tt = np.asarray(time_table, dtype=np.float64)
    base = tt[0]                       # theta_0
    phi = tt[1] - tt[0]                # 0.01*inc
    days = np.arange(N_DAY + 1)[:, None]
    theta = base[None, :] + days * phi[None, :]
    assert np.abs(theta - tt[:N_DAY + 1]).max() < 1e-4
    cs = np.empty((2 * (N_DAY + 1), D), np.float64)
    cs[0::2] = np.cos(theta)
    cs[1::2] = np.sin(theta)

    def ab(w, k):
        """Basis (A,B) for window w given element day k (k>=4 -> generic)."""
        kw = 4 - w
        if k < 4 and w < 4 - k:        # clamped slot -> fixed rotation to 365
            x = (N_DAY - k) * phi
            return np.cos(x), -np.sin(x)
        return np.cos(kw * phi), np.sin(kw * phi)

    def ltab_for(k, wpad):
        # [P, 4(chunk), 3(A,Bp,Bn), wpad]
        tabs = np.zeros((P, 4, 3, wpad), np.float64)
        for w in range(W):
            A, Bv = ab(w, k)
            for c in range(4):
                sl = slice(c * P, (c + 1) * P)
                tabs[:, c, 0, w] = A[sl]
                tabs[:, c, 1, w] = Bv[sl]
                tabs[:, c, 2, w] = -Bv[sl]
        return tabs

    def cblocks(k):
        # per ty: (uA-block, uB'-block) column tables [W, D]
        A = np.zeros((W, D)); Bv = np.zeros((W, D))
        for w in range(W):
            A[w], Bv[w] = ab(w, k)
        # h-side: [uA0|uA1|uB0|uB1'] ; z-side: [uA2'|uA3|uB2'|uB3']
        h = np.zeros((2, W, 4 * D)); z = np.zeros((2, W, 4 * D))
        h[0, :, 0:D] = A;      h[1, :, D:2 * D] = A
        h[0, :, 2 * D:3 * D] = Bv;  h[1, :, 3 * D:4 * D] = -Bv
        z[0, :, 0:D] = -A;     z[1, :, D:2 * D] = A
        z[0, :, 2 * D:3 * D] = -Bv; z[1, :, 3 * D:4 * D] = -Bv
        return h, z

    ltab = ltab_for(4, 32).reshape(P, -1)
    ltabk = np.stack([ltab_for(k, 8) for k in range(4)], axis=1).reshape(P, -1)

    # ctab [P, 4096]: rows 32*ty'+w of the h half (rows 0-63, cols 0:2048)
    # and z half (rows 64-127, cols 2048:4096)
    ctab = np.zeros((P, 4096), np.float64)
    h4, z4 = cblocks(4)
    for u in range(2):
        ctab[32 * u:32 * u + W, 0:2048] = h4[u]
        ctab[64 + 32 * u:64 + 32 * u + W, 2048:4096] = z4[u]
    # ctabk [64, 4096]: row (16k + 8u + w); h cols 0:2048, z cols 2048:4096
    ctabk = np.zeros((64, 4096), np.float64)
    for k in range(4):
        hk, zk = cblocks(k)
        for u in range(2):
            r = slice(16 * k + 8 * u, 16 * k + 8 * u + W)
            ctabk[r, 0:2048] = hk[u]
            ctabk[r, 2048:4096] = zk[u]
    return cs, ltab, ltabk, ctab, ctabk


def _host_prep(h_i, t_i, r_i, d_i, emb_E_real, emb_E_img, emb_R_real,
               emb_R_img, time_table):
    h_i = np.asarray(h_i).astype(np.int64)
    t_i = np.asarray(t_i).astype(np.int64)
    r_i = np.asarray(r_i).astype(np.int64)
    d_i = np.asarray(d_i).astype(np.int64)
    cs, ltab, ltabk, ctab, ctabk = _build_tables(time_table)

    # pair-interleaved mega-table: one gather descriptor -> (real,img) pair
    eE = np.empty((2 * N_ENTITY, D), np.float32)
    eE[0::2] = np.asarray(emb_E_real)
    eE[1::2] = np.asarray(emb_E_img)
    eR = np.empty((2 * N_RELATION, D), np.float32)
    eR[0::2] = np.asarray(emb_R_real)
    eR[1::2] = np.asarray(emb_R_img)
    embM = np.concatenate([eE, eR, cs.astype(np.float32)], axis=0)
    embM = np.ascontiguousarray(embM, dtype=ml_dtypes.bfloat16)
    assert embM.shape == (N_ROWS, D)

    # --- assign elements to (core, slot); clamp elems -> tile-0 k-blocks
    perm = np.full((N_CORES, BL), -1, np.int64)
    clamp = np.where(d_i < 4)[0]
    normal = list(np.where(d_i >= 4)[0])
    kslots = [[[] for _ in range(4)] for _ in range(N_CORES)]
    rr_core = 0
    for b in clamp:
        k = int(d_i[b])
        for off in range(N_CORES):
            c = (rr_core + off) % N_CORES
            if len(kslots[c][k]) < 32:
                kslots[c][k].append(b)
                rr_core = c + 1
                break
        else:
            raise RuntimeError("clamp overflow: >256 elements of one day<4")
    ni = 0
    for c in range(N_CORES):
        for k in range(4):
            blk = kslots[c][k]
            for i in range(32):
                if i < len(blk):
                    perm[c, 32 * k + i] = blk[i]
                else:
                    perm[c, 32 * k + i] = normal[ni]
                    ni += 1
        perm[c, P:] = normal[ni:ni + BL - P]
        ni += BL - P
    assert ni == len(normal) and (perm >= 0).all()

    ident = np.eye(P, dtype=ml_dtypes.bfloat16)
    gsum = np.zeros((P, 4), ml_dtypes.bfloat16)
    grep = np.zeros((4, P), ml_dtypes.bfloat16)
    for ty in range(4):
        gsum[32 * ty:32 * ty + W, ty] = 1
        grep[ty, 32 * ty:32 * (ty + 1)] = 1

    in_maps = []
    perms = []
    for c in range(N_CORES):
        sel = perm[c]
        hh, tt_, rr, dd = h_i[sel], t_i[sel], r_i[sel], d_i[sel]
        cols = np.stack([
            2 * hh, 2 * tt_, ROW_RR + 2 * rr, ROW_CS + 2 * dd],
            axis=1)                                          # [BL, 4]
        idx = np.ascontiguousarray(
            cols.reshape(T, P, 4).transpose(1, 0, 2).reshape(P, T * 4)
        ).astype(np.int32)
        isclamp = (dd[:P] < 4)
        cmask = isclamp.astype(np.uint8).reshape(P, 1)
        lmask = np.zeros((P, P), np.uint8)
        for ty in range(4):
            lmask[32 * ty:32 * ty + W, :] = isclamp[None, :]
        # kmask: row (16k+8u+w) col b -> 1 iff b in k-block [32k,32k+32)
        kmask = np.zeros((64, P), ml_dtypes.bfloat16)
        for k in range(4):
            kmask[16 * k:16 * (k + 1), 32 * k:32 * (k + 1)] = 1
        # sel: cols 0:64 map aln h-rows into k-stack, cols 64:128 the z-rows
        selm = np.zeros((P, P), ml_dtypes.bfloat16)
        for u in range(2):
            for w in range(8):
                for k in range(4):
                    selm[32 * u + w, 16 * k + 8 * u + w] = 1
                    selm[64 + 32 * u + w, 64 + 16 * k + 8 * u + w] = 1
        in_maps.append(dict(
            embM=embM, idx=idx, ident=ident,
            ltab=ltab.astype(ml_dtypes.bfloat16),
            ltabk=ltabk.astype(ml_dtypes.bfloat16),
            ctab=ctab.astype(ml_dtypes.bfloat16),
            ctabk=ctabk.astype(ml_dtypes.bfloat16),
            gsum=gsum, grep=grep, lmask=lmask, cmask=cmask, kmask=kmask,
            sel=selm,
        ))
        perms.append(sel)
    return in_maps, perms


def build_nc():
    nc = bacc.Bacc(
        "TRN2",
        target_bir_lowering=False,
        debug=False,
        enable_asserts=False,
        num_devices=N_CORES,
    )
    ins = dict(
        embM=nc.dram_tensor("embM", [N_ROWS, D], BF16, kind="ExternalInput").ap(),
        idx=nc.dram_tensor("idx", [P, T * 4], I32, kind="ExternalInput").ap(),
        ident=nc.dram_tensor("ident", [P, P], BF16, kind="ExternalInput").ap(),
        ltab=nc.dram_tensor("ltab", [P, 384], BF16, kind="ExternalInput").ap(),
        ltabk=nc.dram_tensor("ltabk", [P, 384], BF16, kind="ExternalInput").ap(),
        ctab=nc.dram_tensor("ctab", [P, 4096], BF16, kind="ExternalInput").ap(),
        ctabk=nc.dram_tensor("ctabk", [64, 4096], BF16,
                             kind="ExternalInput").ap(),
        gsum=nc.dram_tensor("gsum", [P, 4], BF16, kind="ExternalInput").ap(),
        grep=nc.dram_tensor("grep", [4, P], BF16, kind="ExternalInput").ap(),
        lmask=nc.dram_tensor("lmask", [P, P], U8, kind="ExternalInput").ap(),
        cmask=nc.dram_tensor("cmask", [P, 1], U8, kind="ExternalInput").ap(),
        kmask=nc.dram_tensor("kmask", [64, P], BF16, kind="ExternalInput").ap(),
        sel=nc.dram_tensor("sel", [P, P], BF16, kind="ExternalInput").ap(),
    )
    outs = dict(
        out=nc.dram_tensor("out", [P, T], F32, kind="ExternalOutput").ap(),
    )
    with tile.TileContext(nc) as tc:
        _emit(tc, outs, ins)
    nc.compile()
    return nc


_NC_CACHE = {}


def kernel(h_i, t_i, r_i, d_i, emb_E_real, emb_E_img, emb_R_real, emb_R_img,
           time_table, _want_results=False, _trace=False):
    in_maps, perms = _host_prep(h_i, t_i, r_i, d_i, emb_E_real, emb_E_img,
                                emb_R_real, emb_R_img, time_table)
    if "nc" not in _NC_CACHE:
        _NC_CACHE["nc"] = build_nc()
    nc = _NC_CACHE["nc"]
    res = run_bass_kernel_spmd(
        nc, in_maps, core_ids=list(range(N_CORES)), trace=_trace)
    out = np.empty((B,), np.float32)
    for c in range(N_CORES):
        o = np.asarray(res.results[c]["out"])       # [P, T]
        out[perms[c]] = o.T.reshape(BL)             # slot s = t*P + p
    if _want_results:
        return out, res
    return out
